# revision 1
# baseline (speedup 1.0000x reference)
"""DCRNN (PEMS-BAY) Trainium2 Bass kernel, data-parallel over batch on 8 cores.

Transpose-free gconv via S^2 precompute, fp16 matmuls/states, fp32 psum.

Layouts per core (local batch BL=8, split in 2 halves of HB=4):
  A1: [feature partitions, b*384 + n]   (state tiles XH/XR: rows 0:64 = h|rh,
      rows 64:64+din = x)
  B:  [node-chunk partitions (128/128/69), b*Fout + f]  (W-product tiles)
gconv:  pre = X@A0 + S@(X@W1) + S^2@(X@(2*W2))      [A0 = W0 - W2]
  P2B/P1B = direct-to-B W-matmuls (lhsT = XH col-slice, rhs = weight);
  preact accumulated per batch in one psum bank: P0 (lhsT=A0, rhs=XH)
  start=True, then S@P1B + S2@P2B with S/S2 as *rhs* (lhsT = P_B chunk)
  which lands back in A1 layout.  No PE transposes anywhere.
Cand gconv packs 2 batches into 128 partitions ((b%2)*64+f) per psum bank.
"""
import sys
import os
import numpy as np

sys.path.insert(0, "/opt/trn_rl_repo")

import concourse.bass as bass  # noqa: E402
import concourse.mybir as mybir  # noqa: E402
import concourse.tile as tile  # noqa: E402
from concourse import bacc  # noqa: E402
from concourse.bass_utils import run_bass_kernel_spmd  # noqa: E402

# problem constants
N = 325
B = 64
T = 12
HZ = 12
U = 64
DIN = 2
DOUT = 1
NCORES = 8
BL = B // NCORES          # 8 local batch
NB = 384                  # padded node stride per batch
AF = BL * NB              # 3072 A-layout free width
NCH = [(0, 128), (128, 128), (256, 69)]   # node chunks (offset, len)
HB = BL // 2              # 4 batches per half
HAF = HB * NB             # 1536 A cols per half

F32 = mybir.dt.float32
F16 = mybir.dt.float16
AFT = mybir.ActivationFunctionType
ALU = mybir.AluOpType

CELLS = ["enc0", "enc1", "dec0", "dec1"]
CELL_DIN = {"enc0": DIN, "enc1": U, "dec0": DOUT, "dec1": U}

_BUILD_CACHE = {}
LAST_RESULT = None


def _install_ntff_hook():
    """Register the axon NTFF profiling hook if the image lacks antenv.axon_hooks."""
    import types
    import antenv
    if getattr(antenv, "axon_hooks", None) is not None:
        return
    m = types.ModuleType("antenv.axon_hooks")
    state = {"h": None}
    m.set_axon_ntff_profile_hook = lambda h: state.__setitem__("h", h)
    m.get_axon_ntff_profile_hook = lambda: state["h"]
    sys.modules["antenv.axon_hooks"] = m
    antenv.axon_hooks = m
    try:
        from trn_agent_boot.trn_boot import _ntff_profile_via_ctypes
        hook = _ntff_profile_via_ctypes("/opt/axon/libaxon_pjrt.so")
        if hook is not None:
            m.set_axon_ntff_profile_hook(hook)
    except Exception:
        pass


def _pad_w(w, din, fout):
    """(3F, fout) -> three [128, fout] fp16 mats A0, W1, 2*W2.

    Padded row map: rows 0:64 <- h/rh features (orig rows din:F),
    rows 64:64+din <- x features (orig rows 0:din). Others zero.
    """
    f = din + U
    w0, w1, w2 = w[0:f], w[f:2 * f], w[2 * f:3 * f]

    def pad(m):
        p = np.zeros((128, fout), np.float32)
        p[0:64] = m[din:f]
        p[64:64 + din] = m[0:din]
        return p.astype(np.float16)

    return pad(w0 - w2), pad(w1), pad(2.0 * w2)


def _build(nsteps_enc, nsteps_dec):
    key = (nsteps_enc, nsteps_dec)
    if key in _BUILD_CACHE:
        return _BUILD_CACHE[key]

    nc = bacc.Bacc()
    # ---- DRAM params ----
    x_in = nc.declare_dram_parameter("x", [T, DIN, AF], F16, isOutput=False)
    s_in = nc.declare_dram_parameter("s", [3, 128, N], F16, isOutput=False)
    s2_in = nc.declare_dram_parameter("s2", [3, 128, N], F16, isOutput=False)
    wparams = {}
    for c in CELLS:
        for nm, shp, dt_ in [("gA0", [128, 128], F16), ("gW1", [128, 128], F16),
                             ("gW2", [128, 128], F16), ("cA0", [128, 64], F16),
                             ("cW1", [128, 64], F16), ("cW2", [128, 64], F16),
                             ("gb", [128, 1], F32), ("cb2", [128, 1], F32)]:
            wparams[f"{c}_{nm}"] = nc.declare_dram_parameter(
                f"{c}_{nm}", shp, dt_, isOutput=False)
    wparams["pW"] = nc.declare_dram_parameter("pW", [64, 1], F16, isOutput=False)
    wparams["pb128"] = nc.declare_dram_parameter("pb128", [128, 1], F32,
                                                 isOutput=False)
    out_d = nc.declare_dram_parameter("out", [HZ, 1, BL, N], F32, isOutput=True)

    with tile.TileContext(nc) as tc:
        with tc.tile_pool(name="const", bufs=1) as cp, \
             tc.tile_pool(name="state", bufs=1) as st, \
             tc.tile_pool(name="bprod", bufs=1) as bp, \
             tc.tile_pool(name="pstage", bufs=3, space="PSUM") as psp, \
             tc.tile_pool(name="pgate", bufs=3, space="PSUM") as pgp, \
             tc.tile_pool(name="pcand", bufs=2, space="PSUM") as pcp:

            # ---- constants to SBUF ----
            wt = {}
            for c in CELLS:
                for nm in ["gA0", "gW1", "gW2"]:
                    wt[f"{c}_{nm}"] = cp.tile([128, 128], F16, tag=f"{c}_{nm}",
                                              name=f"{c}_{nm}")
                for nm in ["cA0", "cW1", "cW2"]:
                    wt[f"{c}_{nm}"] = cp.tile([128, 64], F16, tag=f"{c}_{nm}",
                                              name=f"{c}_{nm}")
                wt[f"{c}_gb"] = cp.tile([128, 1], F32, tag=f"{c}_gb",
                                        name=f"{c}_gb")
                wt[f"{c}_cb2"] = cp.tile([128, 1], F32, tag=f"{c}_cb2",
                                         name=f"{c}_cb2")
            wt["pW"] = cp.tile([64, 1], F16, tag="pW", name="pW")
            wt["pb128"] = cp.tile([128, 1], F32, tag="pb128", name="pb128")
            for k, t in wt.items():
                nc.sync.dma_start(out=t, in_=wparams[k][:])
            s_t, s2_t = [], []
            for ci, (c0, cl) in enumerate(NCH):
                stl = cp.tile([128, N], F16, tag=f"s{ci}", name=f"s{ci}")
                nc.sync.dma_start(out=stl[0:cl, :], in_=s_in[ci, 0:cl, :])
                s_t.append(stl)
                s2l = cp.tile([128, N], F16, tag=f"s2{ci}", name=f"s2{ci}")
                nc.sync.dma_start(out=s2l[0:cl, :], in_=s2_in[ci, 0:cl, :])
                s2_t.append(s2l)

            # ---- state tiles ----
            xh, xr = {}, {}
            ru, ct, uu = {}, {}, {}
            for hf in range(2):
                for c in CELLS:
                    xh[(c, hf)] = st.tile([128, HAF], F16, tag=f"xh_{c}_{hf}",
                                          name=f"xh_{c}_{hf}")
                    xr[(c, hf)] = st.tile([128, HAF], F16, tag=f"xr_{c}_{hf}",
                                          name=f"xr_{c}_{hf}")
                for lv in range(2):
                    ru[(hf, lv)] = st.tile([128, HAF], F16, tag=f"ru{hf}{lv}",
                                           name=f"ru{hf}{lv}")
                    ct[(hf, lv)] = st.tile([64, HAF], F16, tag=f"ct{hf}{lv}",
                                           name=f"ct{hf}{lv}")
                    uu[(hf, lv)] = st.tile([64, HAF], F16, tag=f"uu{hf}{lv}",
                                           name=f"uu{hf}{lv}")

            projf = {hf: st.tile([1, HAF], F32, tag=f"projf{hf}",
                                 name=f"projf{hf}") for hf in range(2)}

            for tl in (list(xh.values()) + list(xr.values())
                       + list(ru.values()) + list(ct.values())
                       + list(uu.values())):
                nc.vector.memset(tl[:, :], 0.0)
            tc.strict_bb_all_engine_barrier()

            def wprod(cell, hf, src_t, wprefix, fout):
                """Direct-to-B W-matmuls: out[(n-chunk), b*fout+f] tiles.

                Returns 3 SBUF fp16 tiles [cl, HB*fout]. evac engines:
                W2-product on Act, W1-product on Pool.
                """
                w2 = wt[f"{cell}_{wprefix}W2"]
                w1 = wt[f"{cell}_{wprefix}W1"]
                fw = HB * fout
                out = {}
                for role, w_ in (("p2", w2), ("p1", w1)):
                    tiles = []
                    for ci, (c0, cl) in enumerate(NCH):
                        stg = psp.tile([128, 512], F32, tag="stage")
                        for b in range(HB):
                            nc.tensor.matmul(
                                stg[0:cl, b * fout:(b + 1) * fout],
                                src_t[:, b * NB + c0: b * NB + c0 + cl],
                                w_[0:128, 0:fout],
                                start=(b == 0), stop=(b == HB - 1))
                        dst = bp.tile([128, fw], F16,
                                      tag=f"{cell}{wprefix}{role}{ci}_{hf}",
                                      name=f"{cell}{wprefix}{role}{ci}_{hf}")
                        if role == "p2":
                            nc.scalar.copy(dst[0:cl, 0:fw], stg[0:cl, 0:fw])
                        else:
                            nc.vector.tensor_copy(dst[0:cl, 0:fw],
                                                  stg[0:cl, 0:fw])
                        tiles.append(dst)
                    out[role] = tiles
                return out["p2"], out["p1"]

            def cell_phases(cname, hf, mirror_to=None):
                """One DCGRU cell on one batch-half, as 4 phase thunks.

                mirror_to: layer name whose x rows receive h' per batch.
                """
                xh_t = xh[(cname, hf)]
                xr_t = xr[(cname, hf)]
                gb = wt[f"{cname}_gb"]
                cb2 = wt[f"{cname}_cb2"]
                lv = 0 if cname in ("enc0", "dec0") else 1
                ru_t, c_t, uu_t = ru[(hf, lv)], ct[(hf, lv)], uu[(hf, lv)]
                box = {}

                def ph1():     # gate W-products
                    box["g"] = wprod(cname, hf, xh_t, "g", 128)

                def ph2():     # gate psums + sigmoid + rh + uu
                    p2b, p1b = box["g"]
                    a0 = wt[f"{cname}_gA0"]
                    for b in range(HB):
                        pg = pgp.tile([128, 384], F32, tag="pgb")
                        nc.tensor.matmul(pg[0:128, 0:N], a0[0:128, 0:128],
                                         xh_t[:, b * NB:b * NB + N],
                                         start=True, stop=False)
                        for pt, srcs in ((p1b, s_t), (p2b, s2_t)):
                            for k, (c0, cl) in enumerate(NCH):
                                last = pt is p2b and k == 2
                                nc.tensor.matmul(
                                    pg[0:128, 0:N],
                                    pt[k][0:cl, b * 128:(b + 1) * 128],
                                    srcs[k][0:cl, 0:N],
                                    start=False, stop=last)
                        nc.scalar.activation(ru_t[:, b * NB:b * NB + N],
                                             pg[0:128, 0:N], AFT.Sigmoid,
                                             bias=gb[0:128, 0:1])
                        sl = slice(b * NB, (b + 1) * NB)
                        nc.vector.tensor_tensor(xr_t[0:64, sl],
                                                ru_t[0:64, sl],
                                                xh_t[0:64, sl], ALU.mult)
                        nc.vector.tensor_copy(uu_t[0:64, sl],
                                              ru_t[64:128, sl])

                def ph3():     # cand W-products
                    box["c"] = wprod(cname, hf, xr_t, "c", 64)

                def ph4():     # cand psums + tanh + GRU (+ mirror)
                    q2b, q1b = box["c"]
                    ca0 = wt[f"{cname}_cA0"]
                    for pr in range(2):
                        pc = pcp.tile([128, 384], F32, tag="pcb")
                        first = True
                        for pt, srcs in ((q1b, s_t), (q2b, s2_t)):
                            for k, (c0, cl) in enumerate(NCH):
                                nc.tensor.matmul(
                                    pc[0:128, 0:N],
                                    pt[k][0:cl, pr * 128:(pr + 1) * 128],
                                    srcs[k][0:cl, 0:N],
                                    start=first, stop=False)
                                first = False
                        for par in range(2):
                            b = 2 * pr + par
                            nc.tensor.matmul(
                                pc[par * 64:(par + 1) * 64, 0:N],
                                ca0[0:128, 0:64],
                                xr_t[:, b * NB:b * NB + N],
                                start=False, stop=(par == 1))
                        for par in range(2):
                            b = 2 * pr + par
                            sl = slice(b * NB, (b + 1) * NB)
                            nc.scalar.activation(
                                c_t[0:64, b * NB:b * NB + N],
                                pc[par * 64:(par + 1) * 64, 0:N], AFT.Tanh,
                                bias=cb2[par * 64:(par + 1) * 64, 0:1])
                            # GRU: d = h - c -> xr[0:64]; m = u*d; h' = c + m
                            nc.vector.tensor_tensor(xr_t[0:64, sl],
                                                    xh_t[0:64, sl],
                                                    c_t[0:64, sl],
                                                    ALU.subtract)
                            nc.vector.tensor_tensor(xr_t[0:64, sl],
                                                    uu_t[0:64, sl],
                                                    xr_t[0:64, sl], ALU.mult)
                            nc.vector.tensor_tensor(xh_t[0:64, sl],
                                                    c_t[0:64, sl],
                                                    xr_t[0:64, sl], ALU.add)
                            if mirror_to is not None:
                                nc.vector.tensor_copy(
                                    xh[(mirror_to, hf)][64:128, sl],
                                    xh_t[0:64, sl])
                                nc.vector.tensor_copy(
                                    xr[(mirror_to, hf)][64:128, sl],
                                    xh_t[0:64, sl])

                return [ph1, ph2, ph3, ph4]

            def interleave(*phase_lists):
                """Emit phase thunks round-robin: software-pipelines the
                independent cell streams so the PE queue never head-of-line
                blocks on one stream's evac/activation latency."""
                if os.environ.get("DCRNN_NO_PIPELINE"):
                    for pl in phase_lists:
                        for p in pl:
                            p()
                    return
                for i in range(max(len(p) for p in phase_lists)):
                    for pl in phase_lists:
                        if i < len(pl):
                            pl[i]()

            def proj_phase(hf, t):
                """Projection + output DMA + decoder feedback thunk.

                Must be emitted AFTER dec1's ph4 (GRU) — emission order
                defines the dataflow, so an early emit would read stale h.
                """
                def ph():
                    pf = projf[hf]
                    for b in range(HB):
                        pp = psp.tile([1, 384], F32, tag="stage")
                        nc.tensor.matmul(
                            pp[0:1, 0:384],
                            wt["pW"][0:64, 0:1],
                            xh[("dec1", hf)][0:64, b * NB:(b + 1) * NB],
                            start=True, stop=True)
                        nc.scalar.activation(
                            pf[0:1, b * NB:(b + 1) * NB],
                            pp[0:1, 0:384], AFT.Identity,
                            bias=wt["pb128"][0:1, 0:1])
                        if t < nsteps_dec - 1:
                            # feedback: proj -> x rows of dec0
                            nc.vector.tensor_copy(
                                xh[("dec0", hf)][64:65, b * NB:(b + 1) * NB],
                                pf[0:1, b * NB:(b + 1) * NB])
                            nc.vector.tensor_copy(
                                xr[("dec0", hf)][64:65, b * NB:(b + 1) * NB],
                                pf[0:1, b * NB:(b + 1) * NB])
                    ov = pf[0:1, :].rearrange("p (b n) -> p b n", b=HB)
                    nc.sync.dma_start(out=out_d[t][:, hf * HB:(hf + 1) * HB, :],
                                      in_=ov[:, :, 0:N])
                return ph

            # ---- encoder: enc1(t-1) pipelined against enc0(t) ----
            xr3 = x_in[:].rearrange("t d (g f) -> t d g f", g=2)
            prev_l1 = []
            for t in range(nsteps_enc):
                for hf in range(2):
                    nc.sync.dma_start(out=xh[("enc0", hf)][64:66, :],
                                      in_=xr3[t, :, hf, :])
                    nc.sync.dma_start(out=xr[("enc0", hf)][64:66, :],
                                      in_=xr3[t, :, hf, :])
                cur_l0 = [cell_phases("enc0", hf, mirror_to="enc1")
                          for hf in range(2)]
                interleave(*(prev_l1 + cur_l0))
                prev_l1 = [cell_phases("enc1", hf) for hf in range(2)]

            # ---- last enc1 || copy encoder state to decoder ----
            interleave(*prev_l1)
            for hf in range(2):
                nc.vector.tensor_copy(xh[("dec0", hf)][0:64, :],
                                      xh[("enc0", hf)][0:64, :])
                nc.vector.tensor_copy(xh[("dec1", hf)][0:64, :],
                                      xh[("enc1", hf)][0:64, :])

            # ---- decoder (serial: dec0 -> dec1 -> proj feedback) ----
            for t in range(nsteps_dec):
                interleave(*[cell_phases("dec0", hf, mirror_to="dec1")
                             for hf in range(2)])
                d1 = [cell_phases("dec1", hf) for hf in range(2)]
                for hf in range(2):
                    d1[hf].append(proj_phase(hf, t))
                interleave(*d1)

    nc.finalize()
    _BUILD_CACHE[key] = nc
    return nc


def _prep_inputs(inputs, support, weights):
    """Host-side prep. Returns (shared_map, per_core_x list)."""
    s32 = np.asarray(support, np.float32)
    s2_32 = s32 @ s32
    shared = {}
    for nm, m in (("s", s32), ("s2", s2_32)):
        chunks = np.zeros((3, 128, N), np.float16)
        for ci, (c0, cl) in enumerate(NCH):
            chunks[ci, 0:cl, :] = m[c0:c0 + cl, :].astype(np.float16)
        shared[nm] = chunks
    for c in CELLS:
        din = CELL_DIN[c]
        ga0, gw1, gw2 = _pad_w(weights[f"{c}_gate_W"], din, 2 * U)
        ca0, cw1, cw2 = _pad_w(weights[f"{c}_cand_W"], din, U)
        gb = np.zeros((128, 1), np.float32)
        gb[:, 0] = weights[f"{c}_gate_b"]
        cb2 = np.zeros((128, 1), np.float32)
        cb2[0:64, 0] = weights[f"{c}_cand_b"]
        cb2[64:128, 0] = weights[f"{c}_cand_b"]
        shared.update({f"{c}_gA0": ga0, f"{c}_gW1": gw1, f"{c}_gW2": gw2,
                       f"{c}_cA0": ca0, f"{c}_cW1": cw1, f"{c}_cW2": cw2,
                       f"{c}_gb": gb, f"{c}_cb2": cb2})
    shared["pW"] = np.ascontiguousarray(weights["proj_W"]).astype(np.float16)
    pb128 = np.zeros((128, 1), np.float32)
    pb128[:, 0] = float(np.asarray(weights["proj_b"]).reshape(-1)[0])
    shared["pb128"] = pb128

    # inputs (T, B, N*DIN) -> per-core (T, DIN, AF) with node padding
    x = np.asarray(inputs, np.float32).reshape(T, B, N, DIN)
    per_core = []
    for c in range(NCORES):
        xc = x[:, c * BL:(c + 1) * BL]                  # (T, BL, N, DIN)
        xp = np.zeros((T, DIN, BL, NB), np.float16)
        xp[:, :, :, 0:N] = xc.transpose(0, 3, 1, 2)
        per_core.append(xp.reshape(T, DIN, AF))
    return shared, per_core


def kernel(**inputs) -> np.ndarray:
    support = np.asarray(inputs["support"], np.float32)
    weights = {k: np.asarray(v, np.float32) for k, v in inputs.items()
               if k not in ("inputs", "support")}
    shared, per_core_x = _prep_inputs(inputs["inputs"], support, weights)

    nc = _build(T, HZ)
    if os.environ.get("DCRNN_TRACE"):
        _install_ntff_hook()
    in_maps = [dict(shared, x=per_core_x[c]) for c in range(NCORES)]
    res = run_bass_kernel_spmd(nc, in_maps, list(range(NCORES)),
                               trace=bool(os.environ.get("DCRNN_TRACE")))
    global LAST_RESULT
    LAST_RESULT = res
    if res.exec_time_ns is not None:
        print(f"HW exec time: {res.exec_time_ns} ns")
    outs = [res.results[c]["out"].reshape(HZ, BL, N) for c in range(NCORES)]
    return np.concatenate(outs, axis=1).astype(np.float32)


if __name__ == "__main__":
    sys.path.insert(0, "/root/problem")
    import reference
    ins = reference.setup_inputs()
    ins = {k: np.asarray(v) for k, v in ins.items()}
    exp = np.asarray(reference.reference(**ins))
    act = kernel(**ins)
    err = np.max(np.abs(act - exp)) / (np.abs(exp).max() + 1e-30)
    print("Relative error:", err)



# revision 13
# speedup vs baseline: 1.0252x; 1.0252x over previous
"""DCRNN (PEMS-BAY) Trainium2 Bass kernel, data-parallel over batch on 8 cores.

Transpose-free gconv via S^2 precompute, fp16 matmuls/states, fp32 psum
for the diffusion accumulations, fp16 psum staging for the W-products.

Layouts per core (local batch BL=8, split in 2 halves of HB=4):
  A1: [feature partitions, b*NB + n]   (state tiles XH/XR: rows 0:64 = h|rh,
      rows 64:64+din = x)
  B:  [node-chunk partitions (128/128/69), b*F + f]  (W-product tiles)
gconv:  pre = X@A0 + S@(X@W1) + S^2@(X@(2*W2))      [A0 = W0 - W2]
  ph1/ph3 W-products: ONE matmul per (b, chunk) with packed rhs [W1 | 2*W2]
  into fp16 psum, ONE evac copy per chunk into a packed B-tile.
  ph2/ph4: diffusion accumulation per batch in fp32 psum 2-bank pair tiles
  (P0 = a0 start, then S@P1 + S2@P2 with S/S2 as rhs, lhsT = B-tile slices),
  activations applied per PAIR via 3-dim APs spanning both banks.
GRU elementwise merged to [64, HAF] half-wide ops; mirror copies via DMA.
"""
import sys
import os
import numpy as np

sys.path.insert(0, "/opt/trn_rl_repo")

import concourse.bass as bass  # noqa: E402
import concourse.mybir as mybir  # noqa: E402
import concourse.tile as tile  # noqa: E402
from concourse import bacc  # noqa: E402
from concourse.bass_utils import run_bass_kernel_spmd  # noqa: E402

# problem constants
N = 325
B = 64
T = 12
HZ = 12
U = 64
DIN = 2
DOUT = 1
NCORES = 8
BL = B // NCORES          # 8 local batch
NB = 328                  # padded node stride per batch (16B aligned)
AF = BL * NB              # 2624 A-layout free width
NCH = [(0, 128), (128, 128), (256, 69)]   # node chunks (offset, len)
HB = BL // 2              # 4 batches per half
HAF = HB * NB             # 1312 A cols per half

F32 = mybir.dt.float32
F16 = mybir.dt.float16
AFT = mybir.ActivationFunctionType
ALU = mybir.AluOpType

CELLS = ["enc0", "enc1", "dec0", "dec1"]
CELL_DIN = {"enc0": DIN, "enc1": U, "dec0": DOUT, "dec1": U}

_BUILD_CACHE = {}
LAST_RESULT = None


def _install_ntff_hook():
    """Register the axon NTFF profiling hook if the image lacks antenv.axon_hooks."""
    import types
    import antenv
    if getattr(antenv, "axon_hooks", None) is not None:
        return
    m = types.ModuleType("antenv.axon_hooks")
    state = {"h": None}
    m.set_axon_ntff_profile_hook = lambda h: state.__setitem__("h", h)
    m.get_axon_ntff_profile_hook = lambda: state["h"]
    sys.modules["antenv.axon_hooks"] = m
    antenv.axon_hooks = m
    try:
        from trn_agent_boot.trn_boot import _ntff_profile_via_ctypes
        hook = _ntff_profile_via_ctypes("/opt/axon/libaxon_pjrt.so")
        if hook is not None:
            m.set_axon_ntff_profile_hook(hook)
    except Exception:
        pass


def _pad_w(w, din, fout):
    """(3F, fout) -> three [128, fout] fp32 mats A0, W1, 2*W2.

    Padded row map: rows 0:64 <- h/rh features (orig rows din:F),
    rows 64:64+din <- x features (orig rows 0:din). Others zero.
    """
    f = din + U
    w0, w1, w2 = w[0:f], w[f:2 * f], w[2 * f:3 * f]

    def pad(m):
        p = np.zeros((128, fout), np.float32)
        p[0:64] = m[din:f]
        p[64:64 + din] = m[0:din]
        return p

    return pad(w0 - w2), pad(w1), pad(2.0 * w2)


def _build(nsteps_enc, nsteps_dec):
    key = (nsteps_enc, nsteps_dec)
    if key in _BUILD_CACHE:
        return _BUILD_CACHE[key]

    nc = bacc.Bacc()
    # ---- DRAM params ----
    x_in = nc.declare_dram_parameter("x", [T, DIN, AF], F16, isOutput=False)
    s_in = nc.declare_dram_parameter("s", [3, 128, N], F16, isOutput=False)
    s2_in = nc.declare_dram_parameter("s2", [3, 128, N], F16, isOutput=False)
    wparams = {}
    for c in CELLS:
        for nm, shp, dt_ in [("gA0", [128, 128], F16), ("gW12", [128, 256], F16),
                             ("cA0", [128, 64], F16), ("cW12", [128, 128], F16),
                             ("gb", [128, 1], F32), ("cb2", [128, 1], F32)]:
            wparams[f"{c}_{nm}"] = nc.declare_dram_parameter(
                f"{c}_{nm}", shp, dt_, isOutput=False)
    wparams["pW"] = nc.declare_dram_parameter("pW", [64, 1], F16, isOutput=False)
    wparams["pb1"] = nc.declare_dram_parameter("pb1", [1, 1], F32,
                                               isOutput=False)
    out_d = nc.declare_dram_parameter("out", [HZ, 1, BL, N], F16, isOutput=True)

    with tile.TileContext(nc) as tc:
        with tc.tile_pool(name="const", bufs=1) as cp, \
             tc.tile_pool(name="state", bufs=1) as st, \
             tc.tile_pool(name="bprod", bufs=1) as bp, \
             tc.tile_pool(name="pstage", bufs=2, space="PSUM") as psp, \
             tc.tile_pool(name="pacc", bufs=2, space="PSUM") as pap:

            # ---- constants to SBUF ----
            wt = {}
            for c in CELLS:
                wt[f"{c}_gA0"] = cp.tile([128, 128], F16, tag=f"{c}_gA0",
                                         name=f"{c}_gA0")
                wt[f"{c}_gW12"] = cp.tile([128, 256], F16, tag=f"{c}_gW12",
                                          name=f"{c}_gW12")
                wt[f"{c}_cA0"] = cp.tile([128, 64], F16, tag=f"{c}_cA0",
                                         name=f"{c}_cA0")
                wt[f"{c}_cW12"] = cp.tile([128, 128], F16, tag=f"{c}_cW12",
                                          name=f"{c}_cW12")
                wt[f"{c}_gb"] = cp.tile([128, 1], F32, tag=f"{c}_gb",
                                        name=f"{c}_gb")
                wt[f"{c}_cb2"] = cp.tile([128, 1], F32, tag=f"{c}_cb2",
                                         name=f"{c}_cb2")
            wt["pW"] = cp.tile([64, 1], F16, tag="pW", name="pW")
            wt["pb1"] = cp.tile([1, 1], F32, tag="pb1", name="pb1")
            for k, t in wt.items():
                nc.sync.dma_start(out=t, in_=wparams[k][:])
            s_t, s2_t = [], []
            for ci, (c0, cl) in enumerate(NCH):
                stl = cp.tile([128, N], F16, tag=f"s{ci}", name=f"s{ci}")
                nc.sync.dma_start(out=stl[0:cl, :], in_=s_in[ci, 0:cl, :])
                s_t.append(stl)
                s2l = cp.tile([128, N], F16, tag=f"s2{ci}", name=f"s2{ci}")
                nc.sync.dma_start(out=s2l[0:cl, :], in_=s2_in[ci, 0:cl, :])
                s2_t.append(s2l)

            # ---- state tiles ----
            xh, xr = {}, {}
            ru, ct, uu = {}, {}, {}
            for hf in range(2):
                for c in CELLS:
                    xh[(c, hf)] = st.tile([128, HAF], F16, tag=f"xh_{c}_{hf}",
                                          name=f"xh_{c}_{hf}")
                    xr[(c, hf)] = st.tile([128, HAF], F16, tag=f"xr_{c}_{hf}",
                                          name=f"xr_{c}_{hf}")
                for lv in range(2):
                    ru[(hf, lv)] = st.tile([128, HAF], F16, tag=f"ru{hf}{lv}",
                                           name=f"ru{hf}{lv}")
                    ct[(hf, lv)] = st.tile([64, HAF], F16, tag=f"ct{hf}{lv}",
                                           name=f"ct{hf}{lv}")
                    uu[(hf, lv)] = st.tile([64, HAF], F16, tag=f"uu{hf}{lv}",
                                           name=f"uu{hf}{lv}")

            for tl in (list(xh.values()) + list(xr.values())
                       + list(ru.values()) + list(ct.values())
                       + list(uu.values())):
                nc.vector.memset(tl[:, :], 0.0)
            tc.strict_bb_all_engine_barrier()

            def cell_phases(cname, hf, mirror_to=None):
                """One DCGRU cell on one batch-half, as 4 phase thunks.

                mirror_to: layer name whose x rows receive h' per batch.
                """
                xh_t = xh[(cname, hf)]
                xr_t = xr[(cname, hf)]
                gb = wt[f"{cname}_gb"]
                cb2 = wt[f"{cname}_cb2"]
                ga0 = wt[f"{cname}_gA0"]
                gw12 = wt[f"{cname}_gW12"]
                ca0 = wt[f"{cname}_cA0"]
                cw12 = wt[f"{cname}_cW12"]
                lv = 0 if cname in ("enc0", "dec0") else 1
                ru_t, c_t, uu_t = ru[(hf, lv)], ct[(hf, lv)], uu[(hf, lv)]
                box = {}

                def ph1():     # gate W-products (packed W1|2W2, fp16 psum)
                    tiles = []
                    for ci, (c0, cl) in enumerate(NCH):
                        stg = psp.tile([128, 1024], F32, tag="stage")
                        for b in range(HB):
                            # 2-bank tile: each bank's first writer must
                            # carry start=True to clear its has_written bits
                            nc.tensor.matmul(
                                stg[0:cl, b * 256:(b + 1) * 256],
                                xh_t[:, b * NB + c0: b * NB + c0 + cl],
                                gw12[0:128, 0:256],
                                start=(b % 2 == 0), stop=(b % 2 == 1))
                        dst = bp.tile([128, 1024], F16,
                                      tag=f"g{cname}{ci}_{hf}",
                                      name=f"g{cname}{ci}_{hf}")
                        nc.vector.tensor_copy(dst[0:cl, :], stg[0:cl, :])
                        tiles.append(dst)
                    box["g"] = tiles

                def ph2():     # gate psum pairs + sigmoid + rh
                    g = box["g"]
                    for pr in range(2):
                        pg = pap.tile([128, 1024], F32, tag="pacc")
                        for j in range(2):
                            b = 2 * pr + j
                            off = j * 512
                            nc.tensor.matmul(pg[0:128, off:off + N],
                                             ga0[0:128, 0:128],
                                             xh_t[:, b * NB:b * NB + N],
                                             start=True, stop=False)
                            for woff, srcs in ((0, s_t), (128, s2_t)):
                                for k, (c0, cl) in enumerate(NCH):
                                    nc.tensor.matmul(
                                        pg[0:128, off:off + N],
                                        g[k][0:cl,
                                             b * 256 + woff:b * 256 + woff + 128],
                                        srcs[k][0:cl, 0:N],
                                        start=False,
                                        stop=(woff == 128 and k == 2))
                        src = pg[0:128, :].rearrange(
                            "p (j n) -> p j n", n=512)[:, 0:2, 0:N]
                        dst = ru_t[:, :].rearrange(
                            "p (b n) -> p b n", n=NB)[:, 2 * pr:2 * pr + 2, 0:N]
                        nc.scalar.activation(dst, src, AFT.Sigmoid,
                                             bias=gb[0:128, 0:1])
                    # u to partitions 0:64 (two-input DVE ops need equal
                    # base partitions) + r*h
                    nc.scalar.copy(uu_t[0:64, :], ru_t[64:128, :])
                    nc.vector.tensor_tensor(xr_t[0:64, :], ru_t[0:64, :],
                                            xh_t[0:64, :], ALU.mult)

                def ph3():     # cand W-products (packed W1|2W2 matmuls)
                    tiles = []
                    for ci, (c0, cl) in enumerate(NCH):
                        stg = psp.tile([128, 512], F32, tag="stage")
                        for b in range(HB):
                            nc.tensor.matmul(
                                stg[0:cl, b * 128:(b + 1) * 128],
                                xr_t[:, b * NB + c0: b * NB + c0 + cl],
                                cw12[0:128, 0:128],
                                start=(b == 0), stop=(b == HB - 1))
                        # de-interleave roles: b*128+{0:64 w1, 64:128 w2}
                        # -> p1 [cl, 4*64], p2 [cl, 4*64] (pair slices 2D)
                        sv = stg[0:cl, :].rearrange("p (b w f) -> p b w f",
                                                    w=2, f=64)
                        d1 = bp.tile([128, 256], F16, tag=f"c1{cname}{ci}_{hf}",
                                     name=f"c1{cname}{ci}_{hf}")
                        d2 = bp.tile([128, 256], F16, tag=f"c2{cname}{ci}_{hf}",
                                     name=f"c2{cname}{ci}_{hf}")
                        nc.scalar.copy(d1[0:cl, :], sv[:, :, 0, :])
                        nc.vector.tensor_copy(d2[0:cl, :], sv[:, :, 1, :])
                        tiles.append((d1, d2))
                    box["c"] = tiles

                def ph4():     # cand psum (both pairs) + tanh + GRU (+ mirror)
                    q = box["c"]
                    pc = pap.tile([128, 1024], F32, tag="pacc")
                    for pr in range(2):
                        off = pr * 512
                        first = True
                        for role, srcs in ((0, s_t), (1, s2_t)):
                            for k, (c0, cl) in enumerate(NCH):
                                lhs = q[k][role][0:cl,
                                                 pr * 128:(pr + 1) * 128]
                                nc.tensor.matmul(pc[0:128, off:off + N],
                                                 lhs, srcs[k][0:cl, 0:N],
                                                 start=first, stop=False)
                                first = False
                        for par in range(2):
                            b = 2 * pr + par
                            nc.tensor.matmul(
                                pc[par * 64:(par + 1) * 64, off:off + N],
                                ca0[0:128, 0:64],
                                xr_t[:, b * NB:b * NB + N],
                                start=False, stop=(par == 1))
                    for par in range(2):
                        src = pc[par * 64:(par + 1) * 64, :].rearrange(
                            "p (j n) -> p j n", n=512)[:, 0:2, 0:N]
                        dst = c_t[0:64, :].rearrange(
                            "p (g q n) -> p g q n", q=2, n=NB)[:, :, par, 0:N]
                        nc.scalar.activation(dst, src, AFT.Tanh,
                                             bias=cb2[par * 64:(par + 1) * 64,
                                                      0:1])
                    # GRU: d = h - c -> xr; m = u*d; h' = c + m
                    nc.vector.tensor_tensor(xr_t[0:64, :], xh_t[0:64, :],
                                            c_t[0:64, :], ALU.subtract)
                    nc.vector.tensor_tensor(xr_t[0:64, :], uu_t[0:64, :],
                                            xr_t[0:64, :], ALU.mult)
                    nc.vector.tensor_tensor(xh_t[0:64, :], c_t[0:64, :],
                                            xr_t[0:64, :], ALU.add)
                    if mirror_to is not None:
                        nc.sync.dma_start(
                            out=xh[(mirror_to, hf)][64:128, :],
                            in_=xh_t[0:64, :])
                        nc.sync.dma_start(
                            out=xr[(mirror_to, hf)][64:128, :],
                            in_=xh_t[0:64, :])

                return [ph1, ph2, ph3, ph4]

            def interleave(*phase_lists):
                """Emit phase thunks round-robin: software-pipelines the
                independent cell streams so the PE queue never head-of-line
                blocks on one stream's evac/activation latency."""
                if os.environ.get("DCRNN_NO_PIPELINE"):
                    for pl in phase_lists:
                        for p in pl:
                            p()
                    return
                for i in range(max(len(p) for p in phase_lists)):
                    for pl in phase_lists:
                        if i < len(pl):
                            pl[i]()

            def proj_phase(hf, t):
                """Projection writes dec0 x-row directly, + output DMA +
                decoder feedback copy. Emitted AFTER dec1's ph4."""
                def ph():
                    xhd = xh[("dec0", hf)]
                    for pr in range(2):
                        pp = pap.tile([1, 1024], F32, tag="pacc")
                        for j in range(2):
                            b = 2 * pr + j
                            nc.tensor.matmul(
                                pp[0:1, j * 512:j * 512 + NB],
                                wt["pW"][0:64, 0:1],
                                xh[("dec1", hf)][0:64, b * NB:(b + 1) * NB],
                                start=True, stop=True)
                        src = pp[0:1, :].rearrange(
                            "p (j n) -> p j n", n=512)[:, 0:2, 0:NB]
                        dst = xhd[64:65, :].rearrange(
                            "p (b n) -> p b n", n=NB)[:, 2 * pr:2 * pr + 2, :]
                        nc.scalar.activation(dst, src, AFT.Identity,
                                             bias=wt["pb1"][0:1, 0:1])
                    ov = xhd[64:65, :].rearrange("p (b n) -> p b n", n=NB)
                    nc.sync.dma_start(out=out_d[t][:, hf * HB:(hf + 1) * HB, :],
                                      in_=ov[:, :, 0:N])
                    if t < nsteps_dec - 1:
                        nc.vector.tensor_copy(xr[("dec0", hf)][64:65, :],
                                              xhd[64:65, :])
                return ph

            # ---- encoder: enc1(t-1) pipelined against enc0(t) ----
            xr3 = x_in[:].rearrange("t d (g f) -> t d g f", g=2)
            prev_l1 = []
            for t in range(nsteps_enc):
                for hf in range(2):
                    nc.sync.dma_start(out=xh[("enc0", hf)][64:66, :],
                                      in_=xr3[t, :, hf, :])
                    nc.sync.dma_start(out=xr[("enc0", hf)][64:66, :],
                                      in_=xr3[t, :, hf, :])
                cur_l0 = [cell_phases("enc0", hf, mirror_to="enc1")
                          for hf in range(2)]
                interleave(*(prev_l1 + cur_l0))
                prev_l1 = [cell_phases("enc1", hf) for hf in range(2)]

            # ---- last enc1 || copy encoder state to decoder ----
            interleave(*prev_l1)
            for hf in range(2):
                nc.vector.tensor_copy(xh[("dec0", hf)][0:64, :],
                                      xh[("enc0", hf)][0:64, :])
                nc.vector.tensor_copy(xh[("dec1", hf)][0:64, :],
                                      xh[("enc1", hf)][0:64, :])

            # ---- decoder (serial: dec0 -> dec1 -> proj feedback) ----
            for t in range(nsteps_dec):
                interleave(*[cell_phases("dec0", hf, mirror_to="dec1")
                             for hf in range(2)])
                d1 = [cell_phases("dec1", hf) for hf in range(2)]
                for hf in range(2):
                    d1[hf].append(proj_phase(hf, t))
                interleave(*d1)

    nc.finalize()
    _BUILD_CACHE[key] = nc
    return nc


def _prep_inputs(inputs, support, weights):
    """Host-side prep. Returns (shared_map, per_core_x list)."""
    s32 = np.asarray(support, np.float32)
    s2_32 = s32 @ s32
    shared = {}
    for nm, m in (("s", s32), ("s2", s2_32)):
        chunks = np.zeros((3, 128, N), np.float16)
        for ci, (c0, cl) in enumerate(NCH):
            chunks[ci, 0:cl, :] = m[c0:c0 + cl, :].astype(np.float16)
        shared[nm] = chunks
    for c in CELLS:
        din = CELL_DIN[c]
        ga0, gw1, gw2 = _pad_w(weights[f"{c}_gate_W"], din, 2 * U)
        ca0, cw1, cw2 = _pad_w(weights[f"{c}_cand_W"], din, U)
        gb = np.zeros((128, 1), np.float32)
        gb[:, 0] = weights[f"{c}_gate_b"]
        cb2 = np.zeros((128, 1), np.float32)
        cb2[0:64, 0] = weights[f"{c}_cand_b"]
        cb2[64:128, 0] = weights[f"{c}_cand_b"]
        shared.update({
            f"{c}_gA0": ga0.astype(np.float16),
            f"{c}_gW12": np.concatenate([gw1, gw2], 1).astype(np.float16),
            f"{c}_cA0": ca0.astype(np.float16),
            f"{c}_cW12": np.concatenate([cw1, cw2], 1).astype(np.float16),
            f"{c}_gb": gb, f"{c}_cb2": cb2})
    shared["pW"] = np.ascontiguousarray(weights["proj_W"]).astype(np.float16)
    pb1 = np.zeros((1, 1), np.float32)
    pb1[0, 0] = float(np.asarray(weights["proj_b"]).reshape(-1)[0])
    shared["pb1"] = pb1

    # inputs (T, B, N*DIN) -> per-core (T, DIN, AF) with node padding
    x = np.asarray(inputs, np.float32).reshape(T, B, N, DIN)
    per_core = []
    for c in range(NCORES):
        xc = x[:, c * BL:(c + 1) * BL]                  # (T, BL, N, DIN)
        xp = np.zeros((T, DIN, BL, NB), np.float16)
        xp[:, :, :, 0:N] = xc.transpose(0, 3, 1, 2)
        per_core.append(xp.reshape(T, DIN, AF))
    return shared, per_core


def kernel(**inputs) -> np.ndarray:
    support = np.asarray(inputs["support"], np.float32)
    weights = {k: np.asarray(v, np.float32) for k, v in inputs.items()
               if k not in ("inputs", "support")}
    shared, per_core_x = _prep_inputs(inputs["inputs"], support, weights)

    nc = _build(T, HZ)
    if os.environ.get("DCRNN_TRACE"):
        _install_ntff_hook()
    in_maps = [dict(shared, x=per_core_x[c]) for c in range(NCORES)]
    res = run_bass_kernel_spmd(nc, in_maps, list(range(NCORES)),
                               trace=bool(os.environ.get("DCRNN_TRACE")))
    global LAST_RESULT
    LAST_RESULT = res
    if res.exec_time_ns is not None:
        print(f"HW exec time: {res.exec_time_ns} ns")
    outs = [res.results[c]["out"].reshape(HZ, BL, N) for c in range(NCORES)]
    return np.concatenate(outs, axis=1).astype(np.float32)


if __name__ == "__main__":
    sys.path.insert(0, "/root/problem")
    import reference
    ins = reference.setup_inputs()
    ins = {k: np.asarray(v) for k, v in ins.items()}
    exp = np.asarray(reference.reference(**ins))
    act = kernel(**ins)
    err = np.max(np.abs(act - exp)) / (np.abs(exp).max() + 1e-30)
    print("Relative error:", err)


# revision 18
# speedup vs baseline: 1.3155x; 1.2831x over previous
"""DCRNN (PEMS-BAY) Trainium2 Bass kernel, data-parallel over batch on 8 cores.

Transpose-free gconv via S^2 precompute, fp16 matmuls/states, fp32 psum
for the diffusion accumulations, fp16 psum staging for the W-products.

Layouts per core (local batch BL=8, split in 2 halves of HB=4):
  A1: [feature partitions, b*NB + n]   (state tiles XH/XR: rows 0:64 = h|rh,
      rows 64:64+din = x)
  B:  [node-chunk partitions (128/128/69), b*F + f]  (W-product tiles)
gconv:  pre = X@A0 + S@(X@W1) + S^2@(X@(2*W2))      [A0 = W0 - W2]
  ph1/ph3 W-products: ONE matmul per (b, chunk) with packed rhs [W1 | 2*W2]
  into fp16 psum, ONE evac copy per chunk into a packed B-tile.
  ph2/ph4: diffusion accumulation per batch in fp32 psum 2-bank pair tiles
  (P0 = a0 start, then S@P1 + S2@P2 with S/S2 as rhs, lhsT = B-tile slices),
  activations applied per PAIR via 3-dim APs spanning both banks.
GRU elementwise merged to [64, HAF] half-wide ops; mirror copies via DMA.
"""
import sys
import os
import numpy as np

sys.path.insert(0, "/opt/trn_rl_repo")

import concourse.bass as bass  # noqa: E402
import concourse.mybir as mybir  # noqa: E402
import concourse.tile as tile  # noqa: E402
from concourse import bacc  # noqa: E402
from concourse.bass_utils import run_bass_kernel_spmd  # noqa: E402

# problem constants
N = 325
B = 64
T = 12
HZ = 12
U = 64
DIN = 2
DOUT = 1
NCORES = 8
BL = B // NCORES          # 8 local batch
NB = 328                  # padded node stride per batch (16B aligned)
AF = BL * NB              # 2624 A-layout free width
NCH = [(0, 128), (128, 128), (256, 69)]   # node chunks (offset, len)
HB = BL // 2              # 4 batches per half
HAF = HB * NB             # 1312 A cols per half

F32 = mybir.dt.float32
F16 = mybir.dt.float16
AFT = mybir.ActivationFunctionType
ALU = mybir.AluOpType

CELLS = ["enc0", "enc1", "dec0", "dec1"]
CELL_DIN = {"enc0": DIN, "enc1": U, "dec0": DOUT, "dec1": U}

_BUILD_CACHE = {}
LAST_RESULT = None


def _install_ntff_hook():
    """Register the axon NTFF profiling hook if the image lacks antenv.axon_hooks."""
    import types
    import antenv
    if getattr(antenv, "axon_hooks", None) is not None:
        return
    m = types.ModuleType("antenv.axon_hooks")
    state = {"h": None}
    m.set_axon_ntff_profile_hook = lambda h: state.__setitem__("h", h)
    m.get_axon_ntff_profile_hook = lambda: state["h"]
    sys.modules["antenv.axon_hooks"] = m
    antenv.axon_hooks = m
    try:
        from trn_agent_boot.trn_boot import _ntff_profile_via_ctypes
        hook = _ntff_profile_via_ctypes("/opt/axon/libaxon_pjrt.so")
        if hook is not None:
            m.set_axon_ntff_profile_hook(hook)
    except Exception:
        pass


def _pad_w(w, din, fout):
    """(3F, fout) -> three [128, fout] fp32 mats A0, W1, 2*W2.

    Padded row map: rows 0:64 <- h/rh features (orig rows din:F),
    rows 64:64+din <- x features (orig rows 0:din). Others zero.
    """
    f = din + U
    w0, w1, w2 = w[0:f], w[f:2 * f], w[2 * f:3 * f]

    def pad(m):
        p = np.zeros((128, fout), np.float32)
        p[0:64] = m[din:f]
        p[64:64 + din] = m[0:din]
        return p

    return pad(w0 - w2), pad(w1), pad(2.0 * w2)


def _build(nsteps_enc, nsteps_dec):
    key = (nsteps_enc, nsteps_dec)
    if key in _BUILD_CACHE:
        return _BUILD_CACHE[key]

    nc = bacc.Bacc()
    # ---- DRAM params ----
    x_in = nc.declare_dram_parameter("x", [T, DIN, AF], F16, isOutput=False)
    s_in = nc.declare_dram_parameter("s", [3, 128, N], F16, isOutput=False)
    s2_in = nc.declare_dram_parameter("s2", [3, 128, N], F16, isOutput=False)
    wparams = {}
    for c in CELLS:
        for nm, shp, dt_ in [("gA0", [128, 128], F16), ("gW12", [128, 256], F16),
                             ("cA0", [128, 64], F16), ("cW12", [128, 128], F16),
                             ("gb", [128, 1], F32), ("cb2", [128, 1], F32)]:
            wparams[f"{c}_{nm}"] = nc.declare_dram_parameter(
                f"{c}_{nm}", shp, dt_, isOutput=False)
    wparams["pW"] = nc.declare_dram_parameter("pW", [64, 1], F16, isOutput=False)
    wparams["pb1"] = nc.declare_dram_parameter("pb1", [1, 1], F32,
                                               isOutput=False)
    out_d = nc.declare_dram_parameter("out", [HZ, 1, BL, N], F16, isOutput=True)

    with tile.TileContext(nc) as tc:
        with tc.tile_pool(name="const", bufs=1) as cp, \
             tc.tile_pool(name="state", bufs=1) as st, \
             tc.tile_pool(name="bprod", bufs=1) as bp, \
             tc.tile_pool(name="pstage", bufs=4, space="PSUM") as psp, \
             tc.tile_pool(name="pacc", bufs=2, space="PSUM") as pap:

            # ---- constants to SBUF ----
            wt = {}
            for c in CELLS:
                wt[f"{c}_gA0"] = cp.tile([128, 128], F16, tag=f"{c}_gA0",
                                         name=f"{c}_gA0")
                wt[f"{c}_gW12"] = cp.tile([128, 256], F16, tag=f"{c}_gW12",
                                          name=f"{c}_gW12")
                wt[f"{c}_cA0"] = cp.tile([128, 64], F16, tag=f"{c}_cA0",
                                         name=f"{c}_cA0")
                wt[f"{c}_cW12"] = cp.tile([128, 128], F16, tag=f"{c}_cW12",
                                          name=f"{c}_cW12")
                wt[f"{c}_gb"] = cp.tile([128, 1], F32, tag=f"{c}_gb",
                                        name=f"{c}_gb")
                wt[f"{c}_cb2"] = cp.tile([128, 1], F32, tag=f"{c}_cb2",
                                         name=f"{c}_cb2")
            wt["pW"] = cp.tile([64, 1], F16, tag="pW", name="pW")
            wt["pb1"] = cp.tile([1, 1], F32, tag="pb1", name="pb1")
            for k, t in wt.items():
                nc.sync.dma_start(out=t, in_=wparams[k][:])
            s_t, s2_t = [], []
            for ci, (c0, cl) in enumerate(NCH):
                stl = cp.tile([128, N], F16, tag=f"s{ci}", name=f"s{ci}")
                nc.sync.dma_start(out=stl[0:cl, :], in_=s_in[ci, 0:cl, :])
                s_t.append(stl)
                s2l = cp.tile([128, N], F16, tag=f"s2{ci}", name=f"s2{ci}")
                nc.sync.dma_start(out=s2l[0:cl, :], in_=s2_in[ci, 0:cl, :])
                s2_t.append(s2l)

            # ---- state tiles ----
            xh, xr = {}, {}
            ru, ct, uu = {}, {}, {}
            for hf in range(2):
                for c in CELLS:
                    xh[(c, hf)] = st.tile([128, HAF], F16, tag=f"xh_{c}_{hf}",
                                          name=f"xh_{c}_{hf}")
                    xr[(c, hf)] = st.tile([128, HAF], F16, tag=f"xr_{c}_{hf}",
                                          name=f"xr_{c}_{hf}")
                for lv in range(2):
                    ru[(hf, lv)] = st.tile([128, HAF], F16, tag=f"ru{hf}{lv}",
                                           name=f"ru{hf}{lv}")
                    ct[(hf, lv)] = st.tile([64, HAF], F16, tag=f"ct{hf}{lv}",
                                           name=f"ct{hf}{lv}")
                    uu[(hf, lv)] = st.tile([64, HAF], F16, tag=f"uu{hf}{lv}",
                                           name=f"uu{hf}{lv}")

            for tl in (list(xh.values()) + list(xr.values())
                       + list(ru.values()) + list(ct.values())
                       + list(uu.values())):
                nc.vector.memset(tl[:, :], 0.0)
            tc.strict_bb_all_engine_barrier()

            def cell_phases(cname, hf, mirror_to=None):
                """One DCGRU cell on one batch-half, as 4 phase thunks.

                mirror_to: layer name whose x rows receive h' per batch.
                """
                xh_t = xh[(cname, hf)]
                xr_t = xr[(cname, hf)]
                gb = wt[f"{cname}_gb"]
                cb2 = wt[f"{cname}_cb2"]
                ga0 = wt[f"{cname}_gA0"]
                gw12 = wt[f"{cname}_gW12"]
                ca0 = wt[f"{cname}_cA0"]
                cw12 = wt[f"{cname}_cW12"]
                lv = 0 if cname in ("enc0", "dec0") else 1
                ru_t, c_t, uu_t = ru[(hf, lv)], ct[(hf, lv)], uu[(hf, lv)]
                box = {}

                def ph1():     # gate W-products (packed W1|2W2 matmuls)
                    tiles = []
                    for ci, (c0, cl) in enumerate(NCH):
                        dst = bp.tile([128, 1024], F16,
                                      tag=f"g{cname}{ci}_{hf}",
                                      name=f"g{cname}{ci}_{hf}")
                        # per-pair 1-bank stage tiles; evacs alternate
                        # scalar/vector so they run in parallel
                        for pr in range(2):
                            stg = psp.tile([128, 512], F32, tag="stage")
                            for j in range(2):
                                b = 2 * pr + j
                                nc.tensor.matmul(
                                    stg[0:cl, j * 256:(j + 1) * 256],
                                    xh_t[:, b * NB + c0: b * NB + c0 + cl],
                                    gw12[0:128, 0:256],
                                    start=(j == 0), stop=(j == 1))
                            dsl = dst[0:cl, pr * 512:(pr + 1) * 512]
                            if pr == 0:
                                nc.vector.tensor_copy(dsl, stg[0:cl, :])
                            else:
                                nc.scalar.copy(dsl, stg[0:cl, :])
                        tiles.append(dst)
                    box["g"] = tiles

                def ph2():     # gate psum pairs + sigmoid + rh
                    g = box["g"]
                    for pr in range(2):
                        pg = pap.tile([128, 1024], F32, tag="pacc")
                        for j in range(2):
                            b = 2 * pr + j
                            off = j * 512
                            nc.tensor.matmul(pg[0:128, off:off + N],
                                             ga0[0:128, 0:128],
                                             xh_t[:, b * NB:b * NB + N],
                                             start=True, stop=False)
                            for woff, srcs in ((0, s_t), (128, s2_t)):
                                for k, (c0, cl) in enumerate(NCH):
                                    nc.tensor.matmul(
                                        pg[0:128, off:off + N],
                                        g[k][0:cl,
                                             b * 256 + woff:b * 256 + woff + 128],
                                        srcs[k][0:cl, 0:N],
                                        start=False,
                                        stop=(woff == 128 and k == 2))
                        src = pg[0:128, :].rearrange(
                            "p (j n) -> p j n", n=512)[:, 0:2, 0:N]
                        dst = ru_t[:, :].rearrange(
                            "p (b n) -> p b n", n=NB)[:, 2 * pr:2 * pr + 2, 0:N]
                        nc.scalar.activation(dst, src, AFT.Sigmoid,
                                             bias=gb[0:128, 0:1])
                    # u to partitions 0:64 (two-input DVE ops need equal
                    # base partitions) + r*h
                    nc.vector.tensor_copy(uu_t[0:64, :], ru_t[64:128, :])
                    nc.vector.tensor_tensor(xr_t[0:64, :], ru_t[0:64, :],
                                            xh_t[0:64, :], ALU.mult)

                def ph3():     # cand W-products (packed W1|2W2 matmuls)
                    tiles = []
                    for ci, (c0, cl) in enumerate(NCH):
                        stg = psp.tile([128, 512], F32, tag="stage")
                        for b in range(HB):
                            nc.tensor.matmul(
                                stg[0:cl, b * 128:(b + 1) * 128],
                                xr_t[:, b * NB + c0: b * NB + c0 + cl],
                                cw12[0:128, 0:128],
                                start=(b == 0), stop=(b == HB - 1))
                        # de-interleave roles: b*128+{0:64 w1, 64:128 w2}
                        # -> p1 [cl, 4*64], p2 [cl, 4*64] (pair slices 2D)
                        sv = stg[0:cl, :].rearrange("p (b w f) -> p b w f",
                                                    w=2, f=64)
                        d1 = bp.tile([128, 256], F16, tag=f"c1{cname}{ci}_{hf}",
                                     name=f"c1{cname}{ci}_{hf}")
                        d2 = bp.tile([128, 256], F16, tag=f"c2{cname}{ci}_{hf}",
                                     name=f"c2{cname}{ci}_{hf}")
                        nc.scalar.copy(d1[0:cl, :], sv[:, :, 0, :])
                        nc.vector.tensor_copy(d2[0:cl, :], sv[:, :, 1, :])
                        tiles.append((d1, d2))
                    box["c"] = tiles

                def ph4():     # cand psum (both pairs) + tanh + GRU (+ mirror)
                    q = box["c"]
                    pc = pap.tile([128, 1024], F32, tag="pacc")
                    for pr in range(2):
                        off = pr * 512
                        first = True
                        for role, srcs in ((0, s_t), (1, s2_t)):
                            for k, (c0, cl) in enumerate(NCH):
                                lhs = q[k][role][0:cl,
                                                 pr * 128:(pr + 1) * 128]
                                nc.tensor.matmul(pc[0:128, off:off + N],
                                                 lhs, srcs[k][0:cl, 0:N],
                                                 start=first, stop=False)
                                first = False
                        for par in range(2):
                            b = 2 * pr + par
                            nc.tensor.matmul(
                                pc[par * 64:(par + 1) * 64, off:off + N],
                                ca0[0:128, 0:64],
                                xr_t[:, b * NB:b * NB + N],
                                start=False, stop=(par == 1))
                    for par in range(2):
                        src = pc[par * 64:(par + 1) * 64, :].rearrange(
                            "p (j n) -> p j n", n=512)[:, 0:2, 0:N]
                        dst = c_t[0:64, :].rearrange(
                            "p (g q n) -> p g q n", q=2, n=NB)[:, :, par, 0:N]
                        nc.scalar.activation(dst, src, AFT.Tanh,
                                             bias=cb2[par * 64:(par + 1) * 64,
                                                      0:1])
                    # GRU: d = h - c -> xr; m = u*d; h' = c + m
                    nc.vector.tensor_tensor(xr_t[0:64, :], xh_t[0:64, :],
                                            c_t[0:64, :], ALU.subtract)
                    nc.vector.tensor_tensor(xr_t[0:64, :], uu_t[0:64, :],
                                            xr_t[0:64, :], ALU.mult)
                    nc.vector.tensor_tensor(xh_t[0:64, :], c_t[0:64, :],
                                            xr_t[0:64, :], ALU.add)
                    if mirror_to is not None:
                        nc.vector.tensor_copy(
                            xh[(mirror_to, hf)][64:128, :], xh_t[0:64, :])
                        nc.scalar.copy(
                            xr[(mirror_to, hf)][64:128, :], xh_t[0:64, :])

                return [ph1, ph2, ph3, ph4]

            def interleave(*phase_lists):
                """Emit phase thunks round-robin: software-pipelines the
                independent cell streams so the PE queue never head-of-line
                blocks on one stream's evac/activation latency."""
                if os.environ.get("DCRNN_NO_PIPELINE"):
                    for pl in phase_lists:
                        for p in pl:
                            p()
                    return
                for i in range(max(len(p) for p in phase_lists)):
                    for pl in phase_lists:
                        if i < len(pl):
                            pl[i]()

            def proj_phase(hf, t):
                """Projection writes dec0 x-row directly, + output DMA +
                decoder feedback copy. Emitted AFTER dec1's ph4."""
                def ph():
                    xhd = xh[("dec0", hf)]
                    for pr in range(2):
                        pp = pap.tile([1, 1024], F32, tag="pacc")
                        for j in range(2):
                            b = 2 * pr + j
                            nc.tensor.matmul(
                                pp[0:1, j * 512:j * 512 + NB],
                                wt["pW"][0:64, 0:1],
                                xh[("dec1", hf)][0:64, b * NB:(b + 1) * NB],
                                start=True, stop=True)
                        src = pp[0:1, :].rearrange(
                            "p (j n) -> p j n", n=512)[:, 0:2, 0:NB]
                        dst = xhd[64:65, :].rearrange(
                            "p (b n) -> p b n", n=NB)[:, 2 * pr:2 * pr + 2, :]
                        nc.scalar.activation(dst, src, AFT.Identity,
                                             bias=wt["pb1"][0:1, 0:1])
                    ov = xhd[64:65, :].rearrange("p (b n) -> p b n", n=NB)
                    nc.sync.dma_start(out=out_d[t][:, hf * HB:(hf + 1) * HB, :],
                                      in_=ov[:, :, 0:N])
                    if t < nsteps_dec - 1:
                        nc.vector.tensor_copy(xr[("dec0", hf)][64:65, :],
                                              xhd[64:65, :])
                return ph

            # ---- encoder: enc1(t-1) pipelined against enc0(t) ----
            xr3 = x_in[:].rearrange("t d (g f) -> t d g f", g=2)
            prev_l1 = []
            for t in range(nsteps_enc):
                for hf in range(2):
                    nc.sync.dma_start(out=xh[("enc0", hf)][64:66, :],
                                      in_=xr3[t, :, hf, :])
                    nc.sync.dma_start(out=xr[("enc0", hf)][64:66, :],
                                      in_=xr3[t, :, hf, :])
                cur_l0 = [cell_phases("enc0", hf, mirror_to="enc1")
                          for hf in range(2)]
                interleave(*(prev_l1 + cur_l0))
                prev_l1 = [cell_phases("enc1", hf) for hf in range(2)]

            # ---- last enc1 || copy encoder state to decoder ----
            interleave(*prev_l1)
            for hf in range(2):
                nc.vector.tensor_copy(xh[("dec0", hf)][0:64, :],
                                      xh[("enc0", hf)][0:64, :])
                nc.vector.tensor_copy(xh[("dec1", hf)][0:64, :],
                                      xh[("enc1", hf)][0:64, :])

            # ---- decoder (serial: dec0 -> dec1 -> proj feedback) ----
            for t in range(nsteps_dec):
                interleave(*[cell_phases("dec0", hf, mirror_to="dec1")
                             for hf in range(2)])
                d1 = [cell_phases("dec1", hf) for hf in range(2)]
                for hf in range(2):
                    d1[hf].append(proj_phase(hf, t))
                interleave(*d1)

    nc.finalize()
    _BUILD_CACHE[key] = nc
    return nc


def _prep_inputs(inputs, support, weights):
    """Host-side prep. Returns (shared_map, per_core_x list)."""
    s32 = np.asarray(support, np.float32)
    s2_32 = s32 @ s32
    shared = {}
    for nm, m in (("s", s32), ("s2", s2_32)):
        chunks = np.zeros((3, 128, N), np.float16)
        for ci, (c0, cl) in enumerate(NCH):
            chunks[ci, 0:cl, :] = m[c0:c0 + cl, :].astype(np.float16)
        shared[nm] = chunks
    for c in CELLS:
        din = CELL_DIN[c]
        ga0, gw1, gw2 = _pad_w(weights[f"{c}_gate_W"], din, 2 * U)
        ca0, cw1, cw2 = _pad_w(weights[f"{c}_cand_W"], din, U)
        gb = np.zeros((128, 1), np.float32)
        gb[:, 0] = weights[f"{c}_gate_b"]
        cb2 = np.zeros((128, 1), np.float32)
        cb2[0:64, 0] = weights[f"{c}_cand_b"]
        cb2[64:128, 0] = weights[f"{c}_cand_b"]
        shared.update({
            f"{c}_gA0": ga0.astype(np.float16),
            f"{c}_gW12": np.concatenate([gw1, gw2], 1).astype(np.float16),
            f"{c}_cA0": ca0.astype(np.float16),
            f"{c}_cW12": np.concatenate([cw1, cw2], 1).astype(np.float16),
            f"{c}_gb": gb, f"{c}_cb2": cb2})
    shared["pW"] = np.ascontiguousarray(weights["proj_W"]).astype(np.float16)
    pb1 = np.zeros((1, 1), np.float32)
    pb1[0, 0] = float(np.asarray(weights["proj_b"]).reshape(-1)[0])
    shared["pb1"] = pb1

    # inputs (T, B, N*DIN) -> per-core (T, DIN, AF) with node padding
    x = np.asarray(inputs, np.float32).reshape(T, B, N, DIN)
    per_core = []
    for c in range(NCORES):
        xc = x[:, c * BL:(c + 1) * BL]                  # (T, BL, N, DIN)
        xp = np.zeros((T, DIN, BL, NB), np.float16)
        xp[:, :, :, 0:N] = xc.transpose(0, 3, 1, 2)
        per_core.append(xp.reshape(T, DIN, AF))
    return shared, per_core


def kernel(**inputs) -> np.ndarray:
    support = np.asarray(inputs["support"], np.float32)
    weights = {k: np.asarray(v, np.float32) for k, v in inputs.items()
               if k not in ("inputs", "support")}
    shared, per_core_x = _prep_inputs(inputs["inputs"], support, weights)

    nc = _build(T, HZ)
    if os.environ.get("DCRNN_TRACE"):
        _install_ntff_hook()
    in_maps = [dict(shared, x=per_core_x[c]) for c in range(NCORES)]
    res = run_bass_kernel_spmd(nc, in_maps, list(range(NCORES)),
                               trace=bool(os.environ.get("DCRNN_TRACE")))
    global LAST_RESULT
    LAST_RESULT = res
    if res.exec_time_ns is not None:
        print(f"HW exec time: {res.exec_time_ns} ns")
    outs = [res.results[c]["out"].reshape(HZ, BL, N) for c in range(NCORES)]
    return np.concatenate(outs, axis=1).astype(np.float32)


if __name__ == "__main__":
    sys.path.insert(0, "/root/problem")
    import reference
    ins = reference.setup_inputs()
    ins = {k: np.asarray(v) for k, v in ins.items()}
    exp = np.asarray(reference.reference(**ins))
    act = kernel(**ins)
    err = np.max(np.abs(act - exp)) / (np.abs(exp).max() + 1e-30)
    print("Relative error:", err)


# revision 23
# speedup vs baseline: 1.3719x; 1.0429x over previous
"""DCRNN (PEMS-BAY) Trainium2 Bass kernel, data-parallel over batch on 8 cores.

Transpose-free gconv via S^2 precompute, fp16 matmuls/states, fp32 psum
for the diffusion accumulations, fp16 psum staging for the W-products.

Layouts per core (local batch BL=8, split in 2 halves of HB=4):
  A1: [feature partitions, b*NB + n]   (state tiles XH/XR: rows 0:64 = h|rh,
      rows 64:64+din = x)
  B:  [node-chunk partitions (128/128/69), b*F + f]  (W-product tiles)
gconv:  pre = X@A0 + S@(X@W1) + S^2@(X@(2*W2))      [A0 = W0 - W2]
  ph1/ph3 W-products: ONE matmul per (b, chunk) with packed rhs [W1 | 2*W2]
  into fp16 psum, ONE evac copy per chunk into a packed B-tile.
  ph2/ph4: diffusion accumulation per batch in fp32 psum 2-bank pair tiles
  (P0 = a0 start, then S@P1 + S2@P2 with S/S2 as rhs, lhsT = B-tile slices),
  activations applied per PAIR via 3-dim APs spanning both banks.
GRU elementwise merged to [64, HAF] half-wide ops; mirror copies via DMA.
"""
import sys
import os
import numpy as np

sys.path.insert(0, "/opt/trn_rl_repo")

import concourse.bass as bass  # noqa: E402
import concourse.mybir as mybir  # noqa: E402
import concourse.tile as tile  # noqa: E402
from concourse import bacc  # noqa: E402
from concourse.bass_utils import run_bass_kernel_spmd  # noqa: E402

# problem constants
N = 325
B = 64
T = 12
HZ = 12
U = 64
DIN = 2
DOUT = 1
NCORES = 8
BL = B // NCORES          # 8 local batch
NB = 328                  # padded node stride per batch (16B aligned)
AF = BL * NB              # 2624 A-layout free width
NCH = [(0, 128), (128, 128), (256, 69)]   # node chunks (offset, len)
HB = BL // 2              # 4 batches per half
HAF = HB * NB             # 1312 A cols per half

F32 = mybir.dt.float32
F16 = mybir.dt.float16
AFT = mybir.ActivationFunctionType
ALU = mybir.AluOpType

CELLS = ["enc0", "enc1", "dec0", "dec1"]
CELL_DIN = {"enc0": DIN, "enc1": U, "dec0": DOUT, "dec1": U}

_BUILD_CACHE = {}
LAST_RESULT = None


def _install_ntff_hook():
    """Register the axon NTFF profiling hook if the image lacks antenv.axon_hooks."""
    import types
    import antenv
    if getattr(antenv, "axon_hooks", None) is not None:
        return
    m = types.ModuleType("antenv.axon_hooks")
    state = {"h": None}
    m.set_axon_ntff_profile_hook = lambda h: state.__setitem__("h", h)
    m.get_axon_ntff_profile_hook = lambda: state["h"]
    sys.modules["antenv.axon_hooks"] = m
    antenv.axon_hooks = m
    try:
        from trn_agent_boot.trn_boot import _ntff_profile_via_ctypes
        hook = _ntff_profile_via_ctypes("/opt/axon/libaxon_pjrt.so")
        if hook is not None:
            m.set_axon_ntff_profile_hook(hook)
    except Exception:
        pass


def _pad_w(w, din, fout):
    """(3F, fout) -> three [128, fout] fp32 mats A0, W1, 2*W2.

    Padded row map: rows 0:64 <- h/rh features (orig rows din:F),
    rows 64:64+din <- x features (orig rows 0:din). Others zero.
    """
    f = din + U
    w0, w1, w2 = w[0:f], w[f:2 * f], w[2 * f:3 * f]

    def pad(m):
        p = np.zeros((128, fout), np.float32)
        p[0:64] = m[din:f]
        p[64:64 + din] = m[0:din]
        return p

    return pad(w0 - w2), pad(w1), pad(2.0 * w2)


def _build(nsteps_enc, nsteps_dec):
    key = (nsteps_enc, nsteps_dec)
    if key in _BUILD_CACHE:
        return _BUILD_CACHE[key]

    nc = bacc.Bacc()
    # ---- DRAM params ----
    x_in = nc.declare_dram_parameter("x", [T, DIN, AF], F16, isOutput=False)
    s_in = nc.declare_dram_parameter("s", [3, 128, N], F16, isOutput=False)
    s2_in = nc.declare_dram_parameter("s2", [3, 128, N], F16, isOutput=False)
    wparams = {}
    for c in CELLS:
        for nm, shp, dt_ in [("gA0", [128, 128], F16), ("gW12", [128, 256], F16),
                             ("cA0", [128, 64], F16), ("cW12", [128, 128], F16),
                             ("gb", [128, 1], F32), ("cb2", [128, 1], F32)]:
            wparams[f"{c}_{nm}"] = nc.declare_dram_parameter(
                f"{c}_{nm}", shp, dt_, isOutput=False)
    wparams["pW"] = nc.declare_dram_parameter("pW", [64, 1], F16, isOutput=False)
    wparams["pb1"] = nc.declare_dram_parameter("pb1", [1, 1], F32,
                                               isOutput=False)
    out_d = nc.declare_dram_parameter("out", [HZ, 1, BL, N], F16, isOutput=True)

    with tile.TileContext(nc) as tc:
        with tc.tile_pool(name="const", bufs=1) as cp, \
             tc.tile_pool(name="state", bufs=1) as st, \
             tc.tile_pool(name="bprod", bufs=1) as bp, \
             tc.tile_pool(name="pstage", bufs=4, space="PSUM") as psp, \
             tc.tile_pool(name="pacc", bufs=2, space="PSUM") as pap:

            # ---- constants to SBUF ----
            wt = {}
            for c in CELLS:
                wt[f"{c}_gA0"] = cp.tile([128, 128], F16, tag=f"{c}_gA0",
                                         name=f"{c}_gA0")
                wt[f"{c}_gW12"] = cp.tile([128, 256], F16, tag=f"{c}_gW12",
                                          name=f"{c}_gW12")
                wt[f"{c}_cA0"] = cp.tile([128, 64], F16, tag=f"{c}_cA0",
                                         name=f"{c}_cA0")
                wt[f"{c}_cW12"] = cp.tile([128, 128], F16, tag=f"{c}_cW12",
                                          name=f"{c}_cW12")
                wt[f"{c}_gb"] = cp.tile([128, 1], F32, tag=f"{c}_gb",
                                        name=f"{c}_gb")
                wt[f"{c}_cb2"] = cp.tile([128, 1], F32, tag=f"{c}_cb2",
                                         name=f"{c}_cb2")
            wt["pW"] = cp.tile([64, 1], F16, tag="pW", name="pW")
            wt["pb1"] = cp.tile([1, 1], F32, tag="pb1", name="pb1")
            for k, t in wt.items():
                nc.sync.dma_start(out=t, in_=wparams[k][:])
            s_t, s2_t = [], []
            for ci, (c0, cl) in enumerate(NCH):
                stl = cp.tile([128, N], F16, tag=f"s{ci}", name=f"s{ci}")
                nc.sync.dma_start(out=stl[0:cl, :], in_=s_in[ci, 0:cl, :])
                s_t.append(stl)
                s2l = cp.tile([128, N], F16, tag=f"s2{ci}", name=f"s2{ci}")
                nc.sync.dma_start(out=s2l[0:cl, :], in_=s2_in[ci, 0:cl, :])
                s2_t.append(s2l)

            # ---- state tiles ----
            xh, xr = {}, {}
            ru, ct, uu = {}, {}, {}
            for hf in range(2):
                for c in CELLS:
                    xh[(c, hf)] = st.tile([128, HAF], F16, tag=f"xh_{c}_{hf}",
                                          name=f"xh_{c}_{hf}")
                    xr[(c, hf)] = st.tile([128, HAF], F16, tag=f"xr_{c}_{hf}",
                                          name=f"xr_{c}_{hf}")
                for lv in range(2):
                    ru[(hf, lv)] = st.tile([128, HAF], F16, tag=f"ru{hf}{lv}",
                                           name=f"ru{hf}{lv}")
                    ct[(hf, lv)] = st.tile([64, HAF], F16, tag=f"ct{hf}{lv}",
                                           name=f"ct{hf}{lv}")
                    uu[(hf, lv)] = st.tile([64, HAF], F16, tag=f"uu{hf}{lv}",
                                           name=f"uu{hf}{lv}")

            # memsets ordered by first use (enc0 lv0 tiles first) so the
            # first timestep's matmuls are not gated on the whole zero-fill
            ms = []
            for hf in range(2):
                ms += [xh[("enc0", hf)], xr[("enc0", hf)],
                       ru[(hf, 0)], uu[(hf, 0)], ct[(hf, 0)],
                       xh[("enc1", hf)], xr[("enc1", hf)],
                       ru[(hf, 1)], uu[(hf, 1)], ct[(hf, 1)],
                       xh[("dec0", hf)], xr[("dec0", hf)],
                       xh[("dec1", hf)], xr[("dec1", hf)]]
            for tl in ms:
                nc.vector.memset(tl[:, :], 0.0)

            def cell_phases(cname, hf, mirror_to=None):
                """One DCGRU cell on one batch-half, as 4 phase thunks.

                mirror_to: layer name whose x rows receive h' per batch.
                """
                xh_t = xh[(cname, hf)]
                xr_t = xr[(cname, hf)]
                gb = wt[f"{cname}_gb"]
                cb2 = wt[f"{cname}_cb2"]
                ga0 = wt[f"{cname}_gA0"]
                gw12 = wt[f"{cname}_gW12"]
                ca0 = wt[f"{cname}_cA0"]
                cw12 = wt[f"{cname}_cW12"]
                lv = 0 if cname in ("enc0", "dec0") else 1
                ru_t, c_t, uu_t = ru[(hf, lv)], ct[(hf, lv)], uu[(hf, lv)]
                box = {}

                def ph1():     # gate W-products (packed W1|2W2 matmuls)
                    tiles = []
                    for ci, (c0, cl) in enumerate(NCH):
                        dst = bp.tile([128, 1024], F16,
                                      tag=f"g{cname}{ci}_{hf}",
                                      name=f"g{cname}{ci}_{hf}")
                        # per-pair 1-bank stage tiles; evacs alternate
                        # scalar/vector so they run in parallel
                        for pr in range(2):
                            stg = psp.tile([128, 512], F32, tag="stage")
                            for j in range(2):
                                b = 2 * pr + j
                                nc.tensor.matmul(
                                    stg[0:cl, j * 256:(j + 1) * 256],
                                    xh_t[:, b * NB + c0: b * NB + c0 + cl],
                                    gw12[0:128, 0:256],
                                    start=(j == 0), stop=(j == 1))
                            dsl = dst[0:cl, pr * 512:(pr + 1) * 512]
                            if pr == 0:
                                nc.vector.tensor_copy(dsl, stg[0:cl, :])
                            else:
                                nc.scalar.copy(dsl, stg[0:cl, :])
                        tiles.append(dst)
                    box["g"] = tiles

                def ph2():     # gate psum pairs + sigmoid + rh
                    g = box["g"]
                    for pr in range(2):
                        pg = pap.tile([128, 1024], F32, tag="pacc")
                        for j in range(2):
                            b = 2 * pr + j
                            off = j * 512
                            nc.tensor.matmul(pg[0:128, off:off + N],
                                             ga0[0:128, 0:128],
                                             xh_t[:, b * NB:b * NB + N],
                                             start=True, stop=False)
                            for woff, srcs in ((0, s_t), (128, s2_t)):
                                for k, (c0, cl) in enumerate(NCH):
                                    nc.tensor.matmul(
                                        pg[0:128, off:off + N],
                                        g[k][0:cl,
                                             b * 256 + woff:b * 256 + woff + 128],
                                        srcs[k][0:cl, 0:N],
                                        start=False,
                                        stop=(woff == 128 and k == 2))
                        src = pg[0:128, :].rearrange(
                            "p (j n) -> p j n", n=512)[:, 0:2, 0:N]
                        dst = ru_t[:, :].rearrange(
                            "p (b n) -> p b n", n=NB)[:, 2 * pr:2 * pr + 2, 0:N]
                        nc.scalar.activation(dst, src, AFT.Sigmoid,
                                             bias=gb[0:128, 0:1])
                    # u to partitions 0:64 (two-input DVE ops need equal
                    # base partitions) + r*h
                    nc.vector.tensor_copy(uu_t[0:64, :], ru_t[64:128, :])
                    nc.vector.tensor_tensor(xr_t[0:64, :], ru_t[0:64, :],
                                            xh_t[0:64, :], ALU.mult)

                def ph3():     # cand W-products (packed W1|2W2 matmuls)
                    tiles = []
                    for ci, (c0, cl) in enumerate(NCH):
                        stg = psp.tile([128, 512], F32, tag="stage")
                        for b in range(HB):
                            nc.tensor.matmul(
                                stg[0:cl, b * 128:(b + 1) * 128],
                                xr_t[:, b * NB + c0: b * NB + c0 + cl],
                                cw12[0:128, 0:128],
                                start=(b == 0), stop=(b == HB - 1))
                        # de-interleave roles: b*128+{0:64 w1, 64:128 w2}
                        # -> p1 [cl, 4*64], p2 [cl, 4*64] (pair slices 2D)
                        sv = stg[0:cl, :].rearrange("p (b w f) -> p b w f",
                                                    w=2, f=64)
                        d1 = bp.tile([128, 256], F16, tag=f"c1{cname}{ci}_{hf}",
                                     name=f"c1{cname}{ci}_{hf}")
                        d2 = bp.tile([128, 256], F16, tag=f"c2{cname}{ci}_{hf}",
                                     name=f"c2{cname}{ci}_{hf}")
                        nc.scalar.copy(d1[0:cl, :], sv[:, :, 0, :])
                        nc.vector.tensor_copy(d2[0:cl, :], sv[:, :, 1, :])
                        tiles.append((d1, d2))
                    box["c"] = tiles

                def ph4():     # cand psum (both pairs) + tanh + GRU (+ mirror)
                    q = box["c"]
                    pc = pap.tile([128, 1024], F32, tag="pacc")
                    for pr in range(2):
                        off = pr * 512
                        first = True
                        for role, srcs in ((0, s_t), (1, s2_t)):
                            for k, (c0, cl) in enumerate(NCH):
                                lhs = q[k][role][0:cl,
                                                 pr * 128:(pr + 1) * 128]
                                nc.tensor.matmul(pc[0:128, off:off + N],
                                                 lhs, srcs[k][0:cl, 0:N],
                                                 start=first, stop=False)
                                first = False
                        for par in range(2):
                            b = 2 * pr + par
                            nc.tensor.matmul(
                                pc[par * 64:(par + 1) * 64, off:off + N],
                                ca0[0:128, 0:64],
                                xr_t[:, b * NB:b * NB + N],
                                start=False, stop=(par == 1))
                    for par in range(2):
                        src = pc[par * 64:(par + 1) * 64, :].rearrange(
                            "p (j n) -> p j n", n=512)[:, 0:2, 0:N]
                        dst = c_t[0:64, :].rearrange(
                            "p (g q n) -> p g q n", q=2, n=NB)[:, :, par, 0:N]
                        nc.scalar.activation(dst, src, AFT.Tanh,
                                             bias=cb2[par * 64:(par + 1) * 64,
                                                      0:1])

                def ph5():     # GRU elementwise + mirror (no PE work)
                    # GRU: d = h - c -> xr; m = u*d; h' = c + m
                    nc.vector.tensor_tensor(xr_t[0:64, :], xh_t[0:64, :],
                                            c_t[0:64, :], ALU.subtract)
                    nc.vector.tensor_tensor(xr_t[0:64, :], uu_t[0:64, :],
                                            xr_t[0:64, :], ALU.mult)
                    nc.vector.tensor_tensor(xh_t[0:64, :], c_t[0:64, :],
                                            xr_t[0:64, :], ALU.add)
                    if mirror_to is not None:
                        nc.vector.tensor_copy(
                            xh[(mirror_to, hf)][64:128, :], xh_t[0:64, :])
                        nc.scalar.copy(
                            xr[(mirror_to, hf)][64:128, :], xh_t[0:64, :])

                return [ph1, ph2, ph3, ph4, ph5]

            def interleave(*phase_lists):
                """Emit phase thunks round-robin: software-pipelines the
                independent cell streams so the PE queue never head-of-line
                blocks on one stream's evac/activation latency."""
                if os.environ.get("DCRNN_NO_PIPELINE"):
                    for pl in phase_lists:
                        for p in pl:
                            p()
                    return
                for i in range(max(len(p) for p in phase_lists)):
                    for pl in phase_lists:
                        if i < len(pl):
                            pl[i]()

            def proj_phase(hf, t):
                """Projection writes dec0 x-row directly, + output DMA +
                decoder feedback copy. Emitted AFTER dec1's ph4."""
                def ph():
                    xhd = xh[("dec0", hf)]
                    for pr in range(2):
                        pp = pap.tile([1, 1024], F32, tag="pacc")
                        for j in range(2):
                            b = 2 * pr + j
                            nc.tensor.matmul(
                                pp[0:1, j * 512:j * 512 + NB],
                                wt["pW"][0:64, 0:1],
                                xh[("dec1", hf)][0:64, b * NB:(b + 1) * NB],
                                start=True, stop=True)
                        src = pp[0:1, :].rearrange(
                            "p (j n) -> p j n", n=512)[:, 0:2, 0:NB]
                        dst = xhd[64:65, :].rearrange(
                            "p (b n) -> p b n", n=NB)[:, 2 * pr:2 * pr + 2, :]
                        nc.scalar.activation(dst, src, AFT.Identity,
                                             bias=wt["pb1"][0:1, 0:1])
                    ov = xhd[64:65, :].rearrange("p (b n) -> p b n", n=NB)
                    nc.sync.dma_start(out=out_d[t][:, hf * HB:(hf + 1) * HB, :],
                                      in_=ov[:, :, 0:N])
                    if t < nsteps_dec - 1:
                        nc.vector.tensor_copy(xr[("dec0", hf)][64:65, :],
                                              xhd[64:65, :])
                return ph

            # ---- encoder: enc1(t-1) pipelined against enc0(t) ----
            # staggered schedule: each phase sits several emission slots
            # after its producer so no engine queue tail stalls the PE.
            # A = enc0(t) (l0), B = enc1(t-1) (l1); mirror WAR ordering
            # requires B.ph3 before A.ph4.
            xr3 = x_in[:].rearrange("t d (g f) -> t d g f", g=2)
            prev_l1 = None
            for t in range(nsteps_enc):
                for hf in range(2):
                    nc.sync.dma_start(out=xh[("enc0", hf)][64:66, :],
                                      in_=xr3[t, :, hf, :])
                    nc.sync.dma_start(out=xr[("enc0", hf)][64:66, :],
                                      in_=xr3[t, :, hf, :])
                A = [cell_phases("enc0", hf, mirror_to="enc1")
                     for hf in range(2)]
                B = prev_l1
                if B is None:
                    interleave(*A)
                else:
                    # B.ph3/ph4 must precede A.ph5 (A's mirror overwrites
                    # the x rows B's cand matmuls read); A.ph5 sits mid-
                    # round so next round's ph1s never wait a queue tail.
                    for s, i in [(A, 0), (A, 1), (B, 0), (A, 2), (B, 1),
                                 (B, 2), (A, 3), (B, 3), (A, 4), (B, 4)]:
                        for hf in range(2):
                            s[hf][i]()
                prev_l1 = [cell_phases("enc1", hf) for hf in range(2)]

            # ---- last enc1 || copy encoder state to decoder ----
            interleave(*prev_l1)
            for hf in range(2):
                nc.vector.tensor_copy(xh[("dec0", hf)][0:64, :],
                                      xh[("enc0", hf)][0:64, :])
                nc.vector.tensor_copy(xh[("dec1", hf)][0:64, :],
                                      xh[("enc1", hf)][0:64, :])

            # ---- decoder (serial: dec0 -> dec1 -> proj feedback) ----
            # halves staggered so each dependency has PE work in between
            for t in range(nsteps_dec):
                d0 = [cell_phases("dec0", hf, mirror_to="dec1")
                      for hf in range(2)]
                d1 = [cell_phases("dec1", hf) for hf in range(2)]
                pj = [proj_phase(hf, t) for hf in range(2)]
                for s, h, i in [(d0, 0, 0), (d0, 0, 1), (d0, 1, 0),
                                (d0, 0, 2), (d0, 1, 1), (d0, 0, 3),
                                (d0, 1, 2), (d0, 0, 4), (d0, 1, 3),
                                (d0, 1, 4),
                                (d1, 0, 0), (d1, 0, 1), (d1, 1, 0),
                                (d1, 0, 2), (d1, 1, 1), (d1, 0, 3),
                                (d1, 1, 2), (d1, 0, 4), (d1, 1, 3),
                                (pj, 0, None), (d1, 1, 4), (pj, 1, None)]:
                    if i is None:
                        s[h]()
                    else:
                        s[h][i]()

    nc.finalize()
    _BUILD_CACHE[key] = nc
    return nc


def _prep_inputs(inputs, support, weights):
    """Host-side prep. Returns (shared_map, per_core_x list)."""
    s32 = np.asarray(support, np.float32)
    s2_32 = s32 @ s32
    shared = {}
    for nm, m in (("s", s32), ("s2", s2_32)):
        chunks = np.zeros((3, 128, N), np.float16)
        for ci, (c0, cl) in enumerate(NCH):
            chunks[ci, 0:cl, :] = m[c0:c0 + cl, :].astype(np.float16)
        shared[nm] = chunks
    for c in CELLS:
        din = CELL_DIN[c]
        ga0, gw1, gw2 = _pad_w(weights[f"{c}_gate_W"], din, 2 * U)
        ca0, cw1, cw2 = _pad_w(weights[f"{c}_cand_W"], din, U)
        gb = np.zeros((128, 1), np.float32)
        gb[:, 0] = weights[f"{c}_gate_b"]
        cb2 = np.zeros((128, 1), np.float32)
        cb2[0:64, 0] = weights[f"{c}_cand_b"]
        cb2[64:128, 0] = weights[f"{c}_cand_b"]
        shared.update({
            f"{c}_gA0": ga0.astype(np.float16),
            f"{c}_gW12": np.concatenate([gw1, gw2], 1).astype(np.float16),
            f"{c}_cA0": ca0.astype(np.float16),
            f"{c}_cW12": np.concatenate([cw1, cw2], 1).astype(np.float16),
            f"{c}_gb": gb, f"{c}_cb2": cb2})
    shared["pW"] = np.ascontiguousarray(weights["proj_W"]).astype(np.float16)
    pb1 = np.zeros((1, 1), np.float32)
    pb1[0, 0] = float(np.asarray(weights["proj_b"]).reshape(-1)[0])
    shared["pb1"] = pb1

    # inputs (T, B, N*DIN) -> per-core (T, DIN, AF) with node padding
    x = np.asarray(inputs, np.float32).reshape(T, B, N, DIN)
    per_core = []
    for c in range(NCORES):
        xc = x[:, c * BL:(c + 1) * BL]                  # (T, BL, N, DIN)
        xp = np.zeros((T, DIN, BL, NB), np.float16)
        xp[:, :, :, 0:N] = xc.transpose(0, 3, 1, 2)
        per_core.append(xp.reshape(T, DIN, AF))
    return shared, per_core


def kernel(**inputs) -> np.ndarray:
    support = np.asarray(inputs["support"], np.float32)
    weights = {k: np.asarray(v, np.float32) for k, v in inputs.items()
               if k not in ("inputs", "support")}
    shared, per_core_x = _prep_inputs(inputs["inputs"], support, weights)

    nc = _build(T, HZ)
    if os.environ.get("DCRNN_TRACE"):
        _install_ntff_hook()
    in_maps = [dict(shared, x=per_core_x[c]) for c in range(NCORES)]
    res = run_bass_kernel_spmd(nc, in_maps, list(range(NCORES)),
                               trace=bool(os.environ.get("DCRNN_TRACE")))
    global LAST_RESULT
    LAST_RESULT = res
    if res.exec_time_ns is not None:
        print(f"HW exec time: {res.exec_time_ns} ns")
    outs = [res.results[c]["out"].reshape(HZ, BL, N) for c in range(NCORES)]
    return np.concatenate(outs, axis=1).astype(np.float32)


if __name__ == "__main__":
    sys.path.insert(0, "/root/problem")
    import reference
    ins = reference.setup_inputs()
    ins = {k: np.asarray(v) for k, v in ins.items()}
    exp = np.asarray(reference.reference(**ins))
    act = kernel(**ins)
    err = np.max(np.abs(act - exp)) / (np.abs(exp).max() + 1e-30)
    print("Relative error:", err)
